# revision 16
# baseline (speedup 1.0000x reference)
"""Trainium2 Bass kernel for nn_LocalFeatureGuided.

Pipeline per image (C=128 on partitions, spatial on free dim):
  x loaded in 8 row-slices (16 rows each), ring-buffered across images
  BN(eval)+GELU (ACT, fused affine) -> bf16 even/odd column buffers
  depthwise 7x7 s2 conv split by output-row ranges:
    PE (persistent bf16 diag-matmuls, PSUM accum), DVE + GPSIMD
    (scalar_tensor_tensor FMA in bf16)
  tokens: t0=guide (bf16), t1..4 = strided f32r views of x slices
  q0 = WqT.T@t0 ; k_m -> PSUM, consumed by DVE dots -> s_m = <q0,k_m>
  softmax over 5 logits per (b,c); v & proj fused:
    out = sum_m (Wv^T diag(a_m) Pw^T)^T @ t_m  (5 accumulating matmuls)
Sharding: data-parallel over batch, 2 images per core, 8 cores.
"""

import os
import numpy as np
from contextlib import ExitStack

import concourse.bass as bass
import concourse.tile as tile
from concourse import bacc, mybir
from concourse import bass_utils
from concourse import tile_utils

alu = mybir.AluOpType
actf = mybir.ActivationFunctionType
F32 = mybir.dt.float32
F32R = mybir.dt.float32r
BF16 = mybir.dt.bfloat16

B, C, H, W = 16, 128, 128, 128
H2, W2 = H // 2, W // 2
L = H2 * W2            # 4096
NCORES = 8
BPC = B // NCORES      # 2 images per core
EPS = 1e-5
INV_SQRT_C = 1.0 / np.sqrt(128.0)

# ---- tuning knobs ----
DVE_LO = int(os.environ.get("DVE_LO", "20"))  # conv h2 rows [0, DVE_LO) on DVE
# PE takes rows [DVE_LO, 64); DVE rows come first so they only need the
# earliest gelu slices
ACC_BF16 = int(os.environ.get("ACC_BF16", "1"))
CONV_TT = int(os.environ.get("CONV_TT", "1"))  # DVE conv: ts_mul(4x)+tt_add(2x) vs stt(1x)
SBUF_CAP = 204 * 1024

TAPS = [(kh, kw) for kh in range(7) for kw in range(7)]


def g_ap(eo, kh, kw, a, b):
    # full-rect tap read for h2 in [a,b), all w2: row 2*h2+kh in padded
    # row space, col (pad+u)+w2 in the parity buffer
    e = kw - 3
    par, u = (0, e // 2) if e % 2 == 0 else (1, (e - 1) // 2)
    off = (1 + u) if par == 0 else (2 + u)
    return eo[:, par, kh + 2 * a:kh + 2 * b:2, off:off + 64]


def build(nc):
    x_d = nc.dram_tensor("x", (BPC, C, H, W), BF16, kind="ExternalInput").ap()
    bng_d = nc.dram_tensor("bn_gamma", (C, 1), F32, kind="ExternalInput").ap()
    bnb_d = nc.dram_tensor("bn_beta", (C, 1), F32, kind="ExternalInput").ap()
    bnm_d = nc.dram_tensor("bn_mean", (C, 1), F32, kind="ExternalInput").ap()
    bnv_d = nc.dram_tensor("bn_var", (C, 1), F32, kind="ExternalInput").ap()
    dww_d = nc.dram_tensor("dw_w", (C, 49), F32, kind="ExternalInput").ap()
    dwb_d = nc.dram_tensor("dw_b", (C, 1), F32, kind="ExternalInput").ap()
    qkvw_d = nc.dram_tensor("qkv_w", (3 * C, C), F32, kind="ExternalInput").ap()
    qkvb_d = nc.dram_tensor("qkv_b", (3 * C, 1), F32, kind="ExternalInput").ap()
    pw_d = nc.dram_tensor("proj_w", (C, C), F32, kind="ExternalInput").ap()
    pb_d = nc.dram_tensor("proj_b", (C, 1), F32, kind="ExternalInput").ap()
    out_d = nc.dram_tensor("out", (BPC, C, H2, W2), F32, kind="ExternalOutput").ap()

    ACC_DT = BF16 if ACC_BF16 else F32

    with tile.TileContext(nc) as tc, ExitStack() as ctx:
        tp = lambda name, bufs, **kw: ctx.enter_context(
            tc.tile_pool(name=name, bufs=bufs, **kw))

        wpool = tp("weights", 1)       # persistent weights
        eop = tp("eo", 2)              # per-image gelu buffer
        diagp = tp("diag", 1)          # persistent bf16 conv diag tiles
        xp = tp("x", 1)                # x row-slices (per-tile bufs below)
        t0p = tp("t0", 2)
        q0p = tp("q0", 1)
        accp = tp("accd", 2)
        outp = tp("outc", 3)
        vecp = tp("vec", 30)
        emp = tp("em", 2)
        scrp = tp("scr", 1)
        vtp = tp("vt", 2)
        pp512 = tp("pp512", 3, space="PSUM")
        ppk = tp("ppk", 2, space="PSUM")   # [128,1024] = 2 banks each
        ppw = tp("ppw", 1, space="PSUM")

        # ---------- phase 0: weights & per-channel vectors ----------
        def vec_load(src_ap):
            t = vecp.tile([C, 1], F32, tag="v")
            nc.sync.dma_start(t[:], src_ap)
            return t

        gam = vec_load(bng_d)
        bet = vec_load(bnb_d)
        mea = vec_load(bnm_d)
        var = vec_load(bnv_d)
        dwb = vec_load(dwb_d)
        bq = vec_load(qkvb_d[0:C])
        bk = vec_load(qkvb_d[C:2 * C])
        bv = vec_load(qkvb_d[2 * C:3 * C])
        pb = vec_load(pb_d)

        dww = wpool.tile([C, 49], F32)
        nc.sync.dma_start(dww[:], dww_d)

        wq_raw = wpool.tile([C, C], F32)
        nc.sync.dma_start(wq_raw[:], qkvw_d[0:C])
        wk_raw = wpool.tile([C, C], F32)
        nc.sync.dma_start(wk_raw[:], qkvw_d[C:2 * C])
        wv_raw = wpool.tile([C, C], F32)
        nc.sync.dma_start(wv_raw[:], qkvw_d[2 * C:3 * C])
        pw_raw = wpool.tile([C, C], F32)
        nc.sync.dma_start(pw_raw[:], pw_d)
        wvh = wpool.tile([C, C], BF16)     # bf16 lhsT for vts matmuls
        nc.scalar.copy(wvh[:], wv_raw[:])

        # identity for transposes / diag builds
        ones = wpool.tile([C, C], F32)
        nc.gpsimd.memset(ones[:], 1.0)
        ident = wpool.tile([C, C], F32)
        nc.gpsimd.affine_select(ident[:], ones[:], [[-1, C]], alu.is_equal,
                                0.0, base=0, channel_multiplier=1)
        identh = wpool.tile([C, C], BF16)
        nc.scalar.copy(identh[:], ident[:])

        def pe_transpose(src):
            ps = ppw.tile([C, C], F32, tag="w")
            nc.tensor.matmul(ps[:], src[:], ident[:], start=True, stop=True,
                             is_transpose=True)
            return ps

        ps_q = pe_transpose(wq_raw)
        wqTh = wpool.tile([C, C], BF16)
        nc.scalar.copy(wqTh[:], ps_q[:])
        ps_k = pe_transpose(wk_raw)
        wkTh = wpool.tile([C, C], BF16)
        nc.scalar.copy(wkTh[:], ps_k[:])
        ps_p = pe_transpose(pw_raw)
        pwT = wpool.tile([C, C], F32)
        nc.scalar.copy(pwT[:], ps_p[:])

        # BN affine: scale = gamma*rsqrt(var+eps); bias = beta - mean*scale
        epst = vecp.tile([C, 1], F32, tag="v")
        nc.gpsimd.memset(epst[:], EPS)
        std = vecp.tile([C, 1], F32, tag="v")
        nc.scalar.activation(std[:], var[:], actf.Sqrt, bias=epst[:, 0:1],
                             scale=1.0)
        istd = vecp.tile([C, 1], F32, tag="v")
        nc.vector.reciprocal(istd[:], std[:])
        bns = vecp.tile([C, 1], F32, tag="v")
        nc.vector.tensor_tensor(bns[:], gam[:], istd[:], alu.mult)
        bnsn = vecp.tile([C, 1], F32, tag="v")
        nc.vector.tensor_scalar_mul(bnsn[:], bns[:], -1.0)
        bnb = vecp.tile([C, 1], F32, tag="v")
        nc.vector.scalar_tensor_tensor(bnb[:], mea[:], bnsn[:], bet[:],
                                       alu.mult, alu.add)

        # cb = Pw @ bv + pb   (softmax weights sum to 1 -> v-bias folds out)
        cps = ppw.tile([C, 1], F32, tag="w")
        nc.tensor.matmul(cps[:], pwT[:], bv[:], start=True, stop=True)
        cb = vecp.tile([C, 1], F32, tag="v")
        nc.scalar.activation(cb[:], cps[:], actf.Identity, bias=pb[:, 0:1])

        # persistent bf16 diag weight tiles, one per tap
        dts = []
        for (kh, kw) in TAPS:
            t = kh * 7 + kw
            dt_ = diagp.tile([C, C], BF16, tag=f"d{t}")
            nc.vector.tensor_scalar_mul(dt_[:], identh[:], dww[:, t:t + 1])
            dts.append(dt_)

        # ---------- per image ----------
        for img in range(BPC):
            # bf16 zero-padded even/odd column gelu buffer (per image):
            #   E[3+r, 1+j] = g[r, 2j]   (width 68, cols 0 and 65.. are pad)
            #   O[3+r, 2+j] = g[r, 2j+1] (cols 0,1 and 66.. are pad)
            eo = eop.tile([C, 2, 134, 68], BF16)
            nc.gpsimd.memset(eo[:, :, 0:3], 0.0)
            nc.gpsimd.memset(eo[:, :, 131:134], 0.0)
            nc.gpsimd.memset(eo[:, 0, 3:131, 0:1], 0.0)
            nc.gpsimd.memset(eo[:, 0, 3:131, 65:68], 0.0)
            nc.gpsimd.memset(eo[:, 1, 3:131, 0:2], 0.0)
            nc.gpsimd.memset(eo[:, 1, 3:131, 66:68], 0.0)
            # x in 8 row-slice tiles (16 rows each); ring buffer lets the
            # next image's slices prefetch while this image still computes
            xs = []
            for s in range(8):
                xt = xp.tile([C, 16 * W], BF16, tag=f"xs{s}",
                             bufs=2 if s < 6 else 1)
                src = x_d[img, :, 16 * s:16 * (s + 1)].rearrange(
                    "c h w -> c (h w)")
                nc.sync.dma_start(xt[:, 0:1024], src[:, 0:1024])
                nc.sync.dma_start(xt[:, 1024:2048], src[:, 1024:2048])
                xs.append(xt)
            x3 = [t[:].rearrange("c (h w) -> c h w", h=16) for t in xs]

            # BN+GELU into the padded E/O buffers (bf16)
            for s in range(8):
                nc.scalar.activation(
                    eo[:, 0, 3 + 16 * s:19 + 16 * s, 1:65],
                    x3[s][:, :, 0::2], actf.Gelu,
                    bias=bnb[:, 0:1], scale=bns[:, 0:1])
                nc.scalar.activation(
                    eo[:, 1, 3 + 16 * s:19 + 16 * s, 2:66],
                    x3[s][:, :, 1::2], actf.Gelu,
                    bias=bnb[:, 0:1], scale=bns[:, 0:1])

            oi = out_d[img].rearrange("c h w -> c (h w)")
            t0 = t0p.tile([C, L], BF16)

            # --- conv: DVE part (rows [0, DVE_LO)) ---
            nr = DVE_LO
            if nr > 0:
                acc = accp.tile([C, nr, 64], ACC_DT)
                for i, (kh, kw) in enumerate(TAPS):
                    t = kh * 7 + kw
                    g = g_ap(eo, kh, kw, 0, DVE_LO)
                    if CONV_TT:
                        if i == 0:
                            nc.vector.tensor_scalar_mul(acc[:], g, dww[:, 0:1])
                        else:
                            prod = accp.tile([C, nr, 64], ACC_DT, tag="prod",
                                             bufs=2)
                            nc.vector.tensor_scalar_mul(prod[:], g,
                                                        dww[:, t:t + 1])
                            nc.vector.tensor_tensor(acc[:], prod[:], acc[:],
                                                    alu.add)
                    else:
                        if i == 0:
                            nc.vector.memset(acc[:], 0.0)
                        nc.vector.scalar_tensor_tensor(
                            acc[:], g, dww[:, t:t + 1], acc[:],
                            alu.mult, alu.add)
                nc.scalar.activation(
                    t0[:, 0:DVE_LO * 64],
                    acc[:].rearrange("c h w -> c (h w)"),
                    actf.Identity, bias=dwb[:, 0:1])

            # --- conv: PE part (h2 rows [DVE_LO, 64)), 8-row chunks ---
            r0 = DVE_LO
            while r0 < 64:
                r1 = min(r0 + 8, 64)
                ps = pp512.tile([C, (r1 - r0) * 64], F32)
                for i, (kh, kw) in enumerate(TAPS):
                    nc.tensor.matmul(ps[:], dts[i][:], g_ap(eo, kh, kw, r0, r1),
                                     start=(i == 0), stop=(i == 48))
                nc.scalar.activation(t0[:, r0 * 64:r1 * 64], ps[:],
                                     actf.Identity, bias=dwb[:, 0:1])
                r0 = r1

            # token views: m=0 guide (bf16), m>=1 strided f32r views of x
            def tok_x(m, s):
                p, q = (m - 1) // 2, (m - 1) % 2
                return x3[s][:, p::2, q::2]

            # --- q0 ---
            q0 = q0p.tile([C, L], F32)
            q0sums = vecp.tile([C, 8], F32, tag="q0s")
            for ch in range(8):
                ps = pp512.tile([C, 512], F32)
                nc.tensor.matmul(ps[:], wqTh[:], t0[:, ch * 512:(ch + 1) * 512],
                                 start=True, stop=True)
                nc.scalar.activation(q0[:, ch * 512:(ch + 1) * 512], ps[:],
                                     actf.Identity, bias=bq[:, 0:1],
                                     accum_out=q0sums[:, ch:ch + 1])

            # --- k_m + dots ---
            dots = vecp.tile([C, 20], F32, tag="dots")
            for m in range(5):
                for hf in range(4):
                    kp = ppk.tile([C, 1024], F32)
                    for j in range(2):
                        c0 = hf * 1024 + j * 512
                        if m == 0:
                            nc.tensor.matmul(kp[:, j * 512:(j + 1) * 512],
                                             wkTh[:], t0[:, c0:c0 + 512],
                                             start=True, stop=True)
                        else:
                            nc.tensor.matmul(kp[:, j * 512:(j + 1) * 512],
                                             wkTh[:], tok_x(m, 2 * hf + j),
                                             start=True, stop=True)
                    scr = scrp.tile([C, 1024], F32, tag="s")
                    nc.vector.scalar_tensor_tensor(
                        scr[:], q0[:, hf * 1024:(hf + 1) * 1024], 1.0, kp[:],
                        alu.mult, alu.mult,
                        accum_out=dots[:, m * 4 + hf:m * 4 + hf + 1])

            # --- softmax over 5 logits ---
            s5 = vecp.tile([C, 5], F32, tag="s5")
            nc.vector.tensor_reduce(
                s5[:], dots[:].rearrange("c (m h) -> c m h", m=5),
                mybir.AxisListType.X, alu.add)
            q0s = vecp.tile([C, 1], F32, tag="v")
            nc.vector.tensor_reduce(q0s[:], q0sums[:], mybir.AxisListType.X,
                                    alu.add)
            bkq = vecp.tile([C, 1], F32, tag="v")
            nc.vector.tensor_tensor(bkq[:], bk[:], q0s[:], alu.mult)
            nc.vector.tensor_tensor(s5[:], s5[:],
                                    bkq[:, 0:1].broadcast_to((C, 5)), alu.add)
            mx = vecp.tile([C, 1], F32, tag="v")
            nc.vector.tensor_reduce(mx[:], s5[:], mybir.AxisListType.X, alu.max)
            nmx = vecp.tile([C, 1], F32, tag="v")
            nc.vector.tensor_scalar_mul(nmx[:], mx[:], -INV_SQRT_C)
            e5 = vecp.tile([C, 5], F32, tag="s5")
            nc.scalar.activation(e5[:], s5[:], actf.Exp, bias=nmx[:, 0:1],
                                 scale=INV_SQRT_C)
            ssum = vecp.tile([C, 1], F32, tag="v")
            nc.vector.tensor_reduce(ssum[:], e5[:], mybir.AxisListType.X,
                                    alu.add)
            sinv = vecp.tile([C, 1], F32, tag="v")
            nc.vector.reciprocal(sinv[:], ssum[:])
            a5 = vecp.tile([C, 5], F32, tag="s5")
            nc.vector.tensor_scalar_mul(a5[:], e5[:], sinv[:, 0:1])

            # --- fused v+proj: lhsT_m = Wv^T diag(a_m) Pw^T ---
            vts = []
            for m in range(5):
                em = emp.tile([C, C], BF16, tag="em")
                nc.vector.tensor_scalar_mul(em[:], pwT[:], a5[:, m:m + 1])
                vp = ppw.tile([C, C], F32, tag="w")
                nc.tensor.matmul(vp[:], wvh[:], em[:], start=True, stop=True)
                vt = vtp.tile([C, C], BF16, tag=f"vt{m}")
                nc.scalar.copy(vt[:], vp[:])
                vts.append(vt)

            for ch in range(8):
                ps = pp512.tile([C, 512], F32)
                nc.tensor.matmul(ps[:], vts[0][:], t0[:, ch * 512:(ch + 1) * 512],
                                 start=True, stop=False)
                for m in range(1, 5):
                    nc.tensor.matmul(ps[:], vts[m][:],
                                     tok_x(m, ch), start=False, stop=(m == 4))
                oc = outp.tile([C, 512], F32, tag="oc")
                nc.scalar.activation(oc[:], ps[:], actf.Identity,
                                     bias=cb[:, 0:1])
                nc.scalar.dma_start(oi[:, ch * 512:(ch + 1) * 512], oc[:])
    return nc


_CACHE = {}
TRACE = False
LAST_RESULT = None


def _get_nc():
    if "nc" not in _CACHE:
        tile_utils.max_sbuf_usage = SBUF_CAP
        nc = bacc.Bacc("TRN2", target_bir_lowering=False, debug=False,
                       num_devices=NCORES)
        build(nc)
        nc.compile()
        _CACHE["nc"] = nc
    return _CACHE["nc"]


def kernel(x, bn_gamma, bn_beta, bn_mean, bn_var, dw_w, dw_b, qkv_w, qkv_b,
           proj_w, proj_b):
    nc = _get_nc()
    shared = {
        "bn_gamma": np.asarray(bn_gamma, np.float32).reshape(C, 1),
        "bn_beta": np.asarray(bn_beta, np.float32).reshape(C, 1),
        "bn_mean": np.asarray(bn_mean, np.float32).reshape(C, 1),
        "bn_var": np.asarray(bn_var, np.float32).reshape(C, 1),
        "dw_w": np.asarray(dw_w, np.float32).reshape(C, 49),
        "dw_b": np.asarray(dw_b, np.float32).reshape(C, 1),
        "qkv_w": np.asarray(qkv_w, np.float32).reshape(3 * C, C),
        "qkv_b": np.asarray(qkv_b, np.float32).reshape(3 * C, 1),
        "proj_w": np.asarray(proj_w, np.float32).reshape(C, C),
        "proj_b": np.asarray(proj_b, np.float32).reshape(C, 1),
    }
    xf = np.ascontiguousarray(
        np.asarray(x, np.float32).astype(mybir.dt.np(mybir.dt.bfloat16)))
    in_maps = [dict(shared, x=xf[i * BPC:(i + 1) * BPC]) for i in range(NCORES)]
    res = bass_utils.run_bass_kernel_spmd(nc, in_maps,
                                          core_ids=list(range(NCORES)),
                                          trace=TRACE)
    global LAST_RESULT
    LAST_RESULT = res
    return np.concatenate([r["out"] for r in res.results], axis=0)
